# revision 1
# baseline (speedup 1.0000x reference)
"""Bass/Tile Trainium2 kernel for nn_BaseConchGS (GNN message passing).

Strategy: data-parallel over the seed batch (B=4096 -> 512 seeds per core on
8 cores).  All tables are replicated in each core's HBM; every gather happens
on-device via indirect DMA, strictly in the HW-supported form: one offset per
destination partition (128 random rows per call).

Descriptor-count minimization: the host zips edge_node_adj + edge_emb into one
"comb" table [E, 66] i32 (cols 0-1 = endpoints, cols 2-65 = embedding bits), so
each edge costs 1 descriptor for (adj+emb) and 2 for the endpoint features.

Layout trick: each gather call j lands its 128 edges one-per-partition
(edge e = j*128 + p at partition p), so the gathered block is directly a
matmul lhsT.  PE then fuses:
  - transpose + pair-mean     (two matmuls with rhs = 0.5*I, PSUM accumulate)
  - seed-mean over 32 edges   (matmul with rhs = G32 group-indicator / 32)
"""

import numpy as np

P = 128  # partitions


def build_nc(cfg):
    """Build the Bass module for one core (SPMD: every core runs this NEFF)."""
    import concourse.bass as bass
    import concourse.mybir as mybir
    import concourse.tile as tile
    from concourse import bacc

    N, E, S = cfg["N"], cfg["E"], cfg["S"]
    BC, D, DE, NMP = cfg["BC"], cfg["D"], cfg["DE"], cfg["NMP"]
    assert S == 32 and D == 128 and DE == 64
    assert BC % P == 0
    NCHUNK = BC // P          # chunks of 128 seeds
    NBLK = S                  # 32 edge-blocks (of 128 edges) per chunk
    CW = 2 + DE               # comb row: u, v, emb[64]
    f32 = mybir.dt.float32
    i32 = mybir.dt.int32

    nc = bacc.Bacc("TRN2", target_bir_lowering=False)

    # ---- DRAM I/O ----------------------------------------------------------
    feats = nc.dram_tensor("feats", [N, D], f32, kind="ExternalInput")
    SW = NMP * S + D          # seedtab row: n2e_0 | n2e_1 | feats bits
    seedtab = nc.dram_tensor("seedtab", [N, SW], i32, kind="ExternalInput")
    comb = [nc.dram_tensor(f"comb_{m}", [E, CW], i32, kind="ExternalInput")
            for m in range(NMP)]
    prep_w = nc.dram_tensor("prep_w", [D, D], f32, kind="ExternalInput")
    ep_w = nc.dram_tensor("ep_w", [NMP, DE, D], f32, kind="ExternalInput")
    wn_self = nc.dram_tensor("wn_self", [NMP, 2, D, D], f32, kind="ExternalInput")
    wn_neigh = nc.dram_tensor("wn_neigh", [NMP, 2, D, D], f32, kind="ExternalInput")
    we_self = nc.dram_tensor("we_self", [NMP, 2, D, D], f32, kind="ExternalInput")
    we_neigh = nc.dram_tensor("we_neigh", [NMP, 2, D, D], f32, kind="ExternalInput")
    ids_blk = nc.dram_tensor("ids_blk", [P, NCHUNK], i32, kind="ExternalInput")
    ident_d = nc.dram_tensor("ident", [P, P], f32, kind="ExternalInput")
    half_d = nc.dram_tensor("half_ident", [P, P], f32, kind="ExternalInput")
    g32_d = nc.dram_tensor("g32", [P, 4], f32, kind="ExternalInput")
    ig32_d = nc.dram_tensor("identg32", [P, P + 4], f32, kind="ExternalInput")

    out_t = nc.dram_tensor("out", [NMP, BC, 2 * D], f32, kind="ExternalOutput")

    Relu = mybir.ActivationFunctionType.Relu
    IOff = bass.IndirectOffsetOnAxis

    with tile.TileContext(nc) as tc:
        with (
            tc.tile_pool(name="wpool", bufs=1) as wp,
            tc.tile_pool(name="gather", bufs=3) as gp,
            tc.tile_pool(name="small", bufs=3) as sp,
            tc.tile_pool(name="persist", bufs=1) as pp,
            tc.tile_pool(name="psB", bufs=3, space="PSUM") as psB,
            tc.tile_pool(name="psP", bufs=1, space="PSUM") as psP,
        ):
            def load_w(dram_ap, shape, dtype, tag):
                t = wp.tile(shape, dtype, tag=tag, name=tag)
                nc.sync.dma_start(out=t[:], in_=dram_ap)
                return t

            idsb = load_w(ids_blk[:, :], [P, NCHUNK], i32, "idsb")
            ident = load_w(ident_d[:, :], [P, P], f32, "ident")
            half_i = load_w(half_d[:, :], [P, P], f32, "half_i")
            g32 = load_w(g32_d[:, :], [P, 4], f32, "g32")
            ig32 = load_w(ig32_d[:, :], [P, P + 4], f32, "ig32")
            prepw = load_w(prep_w[:, :], [D, D], f32, "prepw")

            wns = [[load_w(wn_self[m, l], [D, D], f32, f"wns_{m}_{l}")
                    for l in range(2)] for m in range(NMP)]
            wnn = [[load_w(wn_neigh[m, l], [D, D], f32, f"wnn_{m}_{l}")
                    for l in range(2)] for m in range(NMP)]
            wes = [load_w(we_self[m, 0], [D, D], f32, f"wes_{m}") for m in range(NMP)]
            wen = [load_w(we_neigh[m, 0], [D, D], f32, f"wen_{m}") for m in range(NMP)]
            epw = [load_w(ep_w[m], [DE, D], f32, f"epw_{m}") for m in range(NMP)]

            # ---- shared: one gather/chunk brings n2e rows (both mps) + feats
            st = pp.tile([P, NCHUNK, SW], i32, tag="st", name="st")
            for c in range(NCHUNK):
                nc.gpsimd.indirect_dma_start(
                    out=st[:, c, :], out_offset=None, in_=seedtab[:, :],
                    in_offset=IOff(ap=idsb[:, c:c + 1], axis=0), oob_is_err=False)
            ps_x0 = psP.tile([P, BC], f32, tag="ps_wide", name="ps_x0")
            for c in range(NCHUNK):
                nc.tensor.transpose(
                    out=ps_x0[:, c * P:(c + 1) * P],
                    in_=st[:, c, NMP * S:SW].bitcast(f32), identity=ident[:, :])
            x0rT = pp.tile([P, BC], f32, tag="x0rT", name="x0rT")
            nc.vector.tensor_copy(out=x0rT[:, :], in_=ps_x0[:, :])
            ps_x0T = psP.tile([P, BC], f32, tag="ps_wide", name="ps_x0T")
            for c in range(NCHUNK):
                nc.tensor.matmul(out=ps_x0T[:, c * P:(c + 1) * P], lhsT=prepw[:, :],
                                 rhs=x0rT[:, c * P:(c + 1) * P], start=True, stop=True)
            x0T = pp.tile([P, BC], f32, tag="x0T", name="x0T")
            nc.vector.tensor_copy(out=x0T[:, :], in_=ps_x0T[:, :])

            for m in range(NMP):
                # ---- fold weights: A = epW @ We_self0, Bm = epW @ Wn_neigh0
                ps_t = psB.tile([P, P], f32, tag="ps_blk", name="ps_epwT")
                nc.tensor.transpose(out=ps_t[0:D, 0:DE], in_=epw[m][:, :],
                                    identity=ident[0:DE, 0:DE])
                epwT = sp.tile([P, DE], f32, tag="epwT", name="epwT")
                nc.vector.tensor_copy(out=epwT[:, :], in_=ps_t[0:D, 0:DE])

                ps_a = psB.tile([P, P], f32, tag="ps_blk", name="ps_a")
                nc.tensor.matmul(out=ps_a[0:DE, :], lhsT=epwT[:, :],
                                 rhs=wes[m][:, :], start=True, stop=True)
                a_t = pp.tile([DE, P], f32, tag="a_t", name="a_t")
                nc.vector.tensor_copy(out=a_t[:, :], in_=ps_a[0:DE, :])

                ps_b = psB.tile([P, P], f32, tag="ps_blk", name="ps_b")
                nc.tensor.matmul(out=ps_b[0:DE, :], lhsT=epwT[:, :],
                                 rhs=wnn[m][0][:, :], start=True, stop=True)
                b_t = pp.tile([DE, P], f32, tag="b_t", name="b_t")
                nc.vector.tensor_copy(out=b_t[:, :], in_=ps_b[0:DE, :])

                # PF = prep_W @ We_neigh0  (so h1 uses s directly, no m1)
                ps_pwT = psB.tile([P, P], f32, tag="ps_blk", name="ps_pwT")
                nc.tensor.transpose(out=ps_pwT[:, :], in_=prepw[:, :],
                                    identity=ident[:, :])
                prepwT = sp.tile([P, P], f32, tag="prepwT", name="prepwT")
                nc.vector.tensor_copy(out=prepwT[:, :], in_=ps_pwT[:, :])
                ps_pf = psB.tile([P, P], f32, tag="ps_blk", name="ps_pf")
                nc.tensor.matmul(out=ps_pf[:, :], lhsT=prepwT[:, :],
                                 rhs=wen[m][:, :], start=True, stop=True)
                pf_t = pp.tile([P, P], f32, tag="pf_t", name="pf_t")
                nc.vector.tensor_copy(out=pf_t[:, :], in_=ps_pf[:, :])

                m0T = pp.tile([DE, BC], f32, tag="m0T", name="m0T")
                ps_mh = psP.tile([P, BC], f32, tag="ps_wide", name="ps_mh")

                # ---- shuffle each chunk's edge ids (from the seed table)
                e_ts = []
                for c in range(NCHUNK):
                    # T = blockwise 32x32 transpose of G
                    t_t = gp.tile([P, S], i32, tag="t_t", name="t_t")
                    nc.vector.transpose(out=t_t[:, :],
                                        in_=st[:, c, m * S:(m + 1) * S])
                    # E_blk[32a+r, 8q+t] = T[32q+r, 4t+a]
                    e_t = gp.tile([P, S], i32, tag=f"e_t{c}", name=f"e_t{c}")
                    e_ts.append(e_t)
                    for a in range(4):
                        for q in range(4):
                            nc.vector.tensor_copy(
                                out=e_t[32 * a:32 * a + 32, 8 * q:8 * q + 8],
                                in_=t_t[32 * q:32 * q + 32, a:a + 29:4])

                for c in range(NCHUNK):
                    e_t = e_ts[c]
                    # ---- comb gather: 32 calls -----------------------------
                    cb = gp.tile([P, NBLK, CW], i32, tag="cb", name="cb")
                    for j in range(NBLK):
                        nc.gpsimd.indirect_dma_start(
                            out=cb[:, j, :], out_offset=None, in_=comb[m][:, :],
                            in_offset=IOff(ap=e_t[:, j:j + 1], axis=0), oob_is_err=False)
                    # ---- endpoint feats: 64 calls; pair-sum on DVE ---------
                    xu = gp.tile([P, NBLK, D], f32, tag="xu", name="xu", bufs=2)
                    xv = gp.tile([P, NBLK, D], f32, tag="xv", name="xv", bufs=2)
                    for j in range(NBLK):
                        nc.gpsimd.indirect_dma_start(
                            out=xu[:, j, :], out_offset=None, in_=feats[:, :],
                            in_offset=IOff(ap=cb[:, j, 0:1], axis=0), oob_is_err=False)
                        nc.gpsimd.indirect_dma_start(
                            out=xv[:, j, :], out_offset=None, in_=feats[:, :],
                            in_offset=IOff(ap=cb[:, j, 1:2], axis=0), oob_is_err=False)

                    for j in range(NBLK):
                        eg_j = cb[:, j, 2:2 + DE].bitcast(f32)

                        # sT = 0.5*(feats[u]+feats[v])^T   [D, 128edges]
                        nc.vector.tensor_add(out=xu[:, j, :], in0=xu[:, j, :],
                                             in1=xv[:, j, :])
                        ps_s = psB.tile([P, P], f32, tag="ps_blk", name="ps_s")
                        nc.tensor.matmul(out=ps_s[:, :], lhsT=xu[:, j, :],
                                         rhs=half_i[:, :], start=True, stop=True)
                        sT = sp.tile([P, P], f32, tag="sT", name="sT")
                        nc.vector.tensor_copy(out=sT[:, :], in_=ps_s[:, :])

                        # [egT | m0cols] = eg_block^T @ [I | g32]
                        ps_eg = psB.tile([P, P + 4], f32, tag="ps_ewide",
                                         name="ps_eg", bufs=2)
                        nc.tensor.matmul(out=ps_eg[0:DE, :], lhsT=eg_j,
                                         rhs=ig32[:, :], start=True, stop=True)
                        egT = sp.tile([DE, P], f32, tag="egT", name="egT")
                        nc.scalar.copy(out=egT[:, :], in_=ps_eg[0:DE, 0:P])
                        nc.scalar.copy(
                            out=m0T[:, c * P + 4 * j: c * P + 4 * j + 4],
                            in_=ps_eg[0:DE, P:P + 4])

                        # h1 = relu(eg@A + m1@We_neigh0)  row-major [128, D]
                        ps_h1 = psB.tile([P, P], f32, tag="ps_blk", name="ps_h1")
                        nc.tensor.matmul(out=ps_h1[:, :], lhsT=egT[:, :],
                                         rhs=a_t[:, :], start=True, stop=False)
                        nc.tensor.matmul(out=ps_h1[:, :], lhsT=sT[:, :],
                                         rhs=pf_t[:, :], start=False, stop=True)
                        h1j = sp.tile([P, P], f32, tag="h1j", name="h1j")
                        nc.scalar.activation(out=h1j[:, :], in_=ps_h1[:, :],
                                             func=Relu)

                        # mh contribution: mean32(h1)^T columns
                        nc.tensor.matmul(
                            out=ps_mh[:, c * P + 4 * j: c * P + 4 * j + 4],
                            lhsT=h1j[:, :], rhs=g32[:, :], start=True, stop=True)

                mhT = pp.tile([P, BC], f32, tag="mhT", name="mhT")
                nc.vector.tensor_copy(out=mhT[:, :], in_=ps_mh[:, :])

                # ---- h0T = relu(Wn_s0^T @ x0T + Bm^T @ m0T) ---------------
                ps_h0 = psP.tile([P, BC], f32, tag="ps_wide", name="ps_h0")
                for c in range(NCHUNK):
                    cs = slice(c * P, (c + 1) * P)
                    nc.tensor.matmul(out=ps_h0[:, cs], lhsT=wns[m][0][:, :],
                                     rhs=x0T[:, cs], start=True, stop=False)
                    nc.tensor.matmul(out=ps_h0[:, cs], lhsT=b_t[:, :],
                                     rhs=m0T[:, cs], start=False, stop=True)
                h0T = pp.tile([P, BC], f32, tag="h0T", name="h0T")
                nc.scalar.activation(out=h0T[:, :], in_=ps_h0[:, :], func=Relu)

                # ---- out1T = relu(Wn_s1^T @ h0T + Wn_n1^T @ mhT) ----------
                ps_o1 = psP.tile([P, BC], f32, tag="ps_wide", name="ps_o1")
                for c in range(NCHUNK):
                    cs = slice(c * P, (c + 1) * P)
                    nc.tensor.matmul(out=ps_o1[:, cs], lhsT=wns[m][1][:, :],
                                     rhs=h0T[:, cs], start=True, stop=False)
                    nc.tensor.matmul(out=ps_o1[:, cs], lhsT=wnn[m][1][:, :],
                                     rhs=mhT[:, cs], start=False, stop=True)
                o1T = pp.tile([P, BC], f32, tag="o1T", name="o1T")
                nc.scalar.activation(out=o1T[:, :], in_=ps_o1[:, :], func=Relu)

                # ---- writeback: transpose back to row-major, DMA out ------
                for c in range(NCHUNK):
                    cs = slice(c * P, (c + 1) * P)
                    for src, col0 in ((h0T, 0), (o1T, D)):
                        ps_w = psB.tile([P, P], f32, tag="ps_blk", name="ps_w")
                        nc.tensor.transpose(out=ps_w[:, :], in_=src[:, cs],
                                            identity=ident[:, :])
                        ob = sp.tile([P, P], f32, tag="ob", name="ob")
                        nc.vector.tensor_copy(out=ob[:, :], in_=ps_w[:, :])
                        nc.sync.dma_start(
                            out=out_t[m, c * P:(c + 1) * P, col0:col0 + D],
                            in_=ob[:, :])

    nc.compile()
    return nc


# ----------------------------------------------------------------------------
# Host-side input preparation (sharding + constants)
# ----------------------------------------------------------------------------
def make_in_maps(inputs, cfg, n_cores):
    S, BC, NMP = cfg["S"], cfg["BC"], cfg["NMP"]
    NCHUNK = BC // P

    ids = np.asarray(inputs["ids"]).astype(np.int32)

    common = {
        "feats": np.ascontiguousarray(np.asarray(inputs["feats"], dtype=np.float32)),
        "prep_w": np.asarray(inputs["prep_W"], dtype=np.float32),
        "ep_w": np.asarray(inputs["edge_prep_W"], dtype=np.float32),
        "wn_self": np.asarray(inputs["Wn_self"], dtype=np.float32),
        "wn_neigh": np.asarray(inputs["Wn_neigh"], dtype=np.float32),
        "we_self": np.asarray(inputs["We_self"], dtype=np.float32),
        "we_neigh": np.asarray(inputs["We_neigh"], dtype=np.float32),
        "ident": np.eye(P, dtype=np.float32),
        "half_ident": (0.5 * np.eye(P)).astype(np.float32),
        "g32": np.ascontiguousarray(
            np.repeat(np.eye(4, dtype=np.float32), 32, axis=0) / 32.0),
        "identg32": np.ascontiguousarray(np.concatenate(
            [np.eye(P, dtype=np.float32),
             np.repeat(np.eye(4, dtype=np.float32), 32, axis=0) / 32.0],
            axis=1)),
    }
    common["seedtab"] = np.ascontiguousarray(np.concatenate(
        [np.asarray(inputs["node2edge_idx_0"], dtype=np.int32),
         np.asarray(inputs["node2edge_idx_1"], dtype=np.int32),
         np.asarray(inputs["feats"], dtype=np.float32).view(np.int32)], axis=1))
    for mn in range(NMP):
        adj = np.asarray(inputs[f"edge_node_adj_{mn}"], dtype=np.int32)
        emb = np.ascontiguousarray(
            np.asarray(inputs[f"edge_emb_{mn}"], dtype=np.float32))
        common[f"comb_{mn}"] = np.ascontiguousarray(
            np.concatenate([adj, emb.view(np.int32)], axis=1))

    p_arr = np.arange(P)
    in_maps = []
    for core in range(n_cores):
        shard = ids[core * BC:(core + 1) * BC]
        ids_blk = np.empty((P, NCHUNK), np.int32)
        for c in range(NCHUNK):
            ids_blk[:, c] = shard[c * P + p_arr]
        m = dict(common)
        m["ids_blk"] = ids_blk
        in_maps.append(m)
    return in_maps


def assemble_output(results, cfg, n_cores):
    NMP, BC, D = cfg["NMP"], cfg["BC"], cfg["D"]
    out = np.empty((NMP, n_cores * BC, 2 * D), np.float32)
    for core in range(n_cores):
        out[:, core * BC:(core + 1) * BC, :] = results[core]["out"]
    return out


FULL_CFG = dict(N=100000, E=400000, S=32, BC=512, D=128, DE=64, NMP=2)

_NC_CACHE = {}


def kernel(**inputs) -> np.ndarray:
    import sys
    for path in ("/opt/trn_rl_repo", "/root/.axon_site/_ro/trn_rl_repo"):
        if path not in sys.path:
            sys.path.append(path)
    from concourse.bass_utils import run_bass_kernel_spmd

    cfg = FULL_CFG
    n_cores = 8
    if "full" not in _NC_CACHE:
        _NC_CACHE["full"] = build_nc(cfg)
    nc = _NC_CACHE["full"]
    in_maps = make_in_maps(inputs, cfg, n_cores)
    res = run_bass_kernel_spmd(nc, in_maps, core_ids=list(range(n_cores)))
    return assemble_output(res.results, cfg, n_cores)



# revision 8
# speedup vs baseline: 3.5803x; 3.5803x over previous
"""Bass/Tile Trainium2 kernel for nn_BaseConchGS (GNN message passing).

v4 strategy. Measurement on this hardware shows SWDGE descriptor generation
costs ~8.4ns per descriptor of serial GpSimd time regardless of descriptor
size, so performance is governed by descriptor COUNT, and fat descriptors
are nearly free bandwidth-wise. Therefore:

  - Host builds batch-INDEPENDENT denormalized per-node tables (pure
    relayout of the input tables, no model math): for each node n and each
    of its S=32 incident edges e = node2edge[n, j]:
        slot = [edge_emb[e] as bf16 (128B) | feats[u_e] fp8 (128B)
                | feats[v_e] fp8 (128B)]            -> 384B, row = 12.3KB
    The fp8 endpoint feats only influence the o1 neighbor term (the h1 ->
    mean path), where the error analysis gives ~0.5% - well within 2e-2.
  - Tables are sharded by node range across the 8 cores (N/8 = 12500 rows
    per core per metapath), and each SEED is assigned to the core owning
    its node's rows (data-parallel seed sharding, zero cross-core traffic;
    capacity 640 = mean 512 + 6 sigma per core).
  - Device work per (metapath, chunk-of-128-seeds): ONE indirect gather
    (128 descriptors x 12.3KB), then per edge-slot j: transpose the emb
    block and pair-transpose the fp8 endpoint feats on the PE, h1 in
    512-column groups with stationary folded weights (A = epW @ We_self0,
    PF = prep_W @ We_neigh0, B = epW @ Wn_neigh0 / 32), relu folds the
    1/32 mean, mh/m0 reduced with short DVE add-trees.

Per-core totals: ~1.9k descriptors (~17us Pool), ~16MB gathered (~50us
DMA), PE ~100us => wall well under the 1.1ms baseline.
"""

import numpy as np

P = 128  # partitions


def build_nc(cfg):
    """Build the Bass module for one core (SPMD: every core runs this NEFF)."""
    import concourse.bass as bass
    import concourse.mybir as mybir
    import concourse.tile as tile
    from concourse import bacc

    N, S = cfg["N"], cfg["S"]
    D, DE, NMP = cfg["D"], cfg["DE"], cfg["NMP"]
    NSH = N // 8              # node-shard rows per core
    BCC = cfg["BCC"]          # seed capacity per core
    NCHUNK = BCC // P         # 5 chunks of 128 seeds
    NGRP = S // 4             # 8 groups of 4 slots per chunk
    SLOT = 2 * DE + 2 * D     # 384 bytes: emb bf16 | fu fp8 | fv fp8
    f32 = mybir.dt.float32
    bf16 = mybir.dt.bfloat16
    f8 = mybir.dt.float8e4
    u8 = mybir.dt.uint8
    i32 = mybir.dt.int32

    nc = bacc.Bacc("TRN2", target_bir_lowering=False)

    # ---- DRAM I/O ----------------------------------------------------------
    tab = [nc.dram_tensor(f"tab_{m}", [NSH, S, SLOT], u8, kind="ExternalInput")
           for m in range(NMP)]
    feats_sh = nc.dram_tensor("feats_sh", [NSH, D], f32, kind="ExternalInput")
    ids_loc = nc.dram_tensor("ids_loc", [P, NCHUNK], i32, kind="ExternalInput")
    prep_w = nc.dram_tensor("prep_w", [D, D], f32, kind="ExternalInput")
    ep_w = nc.dram_tensor("ep_w", [NMP, DE, D], f32, kind="ExternalInput")
    we_self = nc.dram_tensor("we_self", [NMP, 2, D, D], f32, kind="ExternalInput")
    wn_neigh = nc.dram_tensor("wn_neigh", [NMP, 2, D, D], f32, kind="ExternalInput")
    we_neigh = nc.dram_tensor("we_neigh", [NMP, 2, D, D], f32, kind="ExternalInput")
    wn_self_bf = nc.dram_tensor("wn_self_bf", [NMP, 2, D, D], bf16,
                                kind="ExternalInput")
    wn_neigh1_bf = nc.dram_tensor("wn_neigh1_bf", [NMP, D, D], bf16,
                                  kind="ExternalInput")
    ident_d = nc.dram_tensor("ident", [P, P], f32, kind="ExternalInput")
    identb_d = nc.dram_tensor("ident_bf", [P, P], bf16, kind="ExternalInput")
    halfi8_d = nc.dram_tensor("half_ident_f8", [P, P], f8, kind="ExternalInput")

    out_t = nc.dram_tensor("out", [NMP, BCC, 2 * D], f32, kind="ExternalOutput")

    Relu = mybir.ActivationFunctionType.Relu
    IOff = bass.IndirectOffsetOnAxis

    with tile.TileContext(nc) as tc:
        with (
            tc.tile_pool(name="wpool", bufs=1) as wp,
            tc.tile_pool(name="gather", bufs=3) as gp,
            tc.tile_pool(name="small", bufs=3) as sp,
            tc.tile_pool(name="tree", bufs=2) as tp,
            tc.tile_pool(name="psS", bufs=2, space="PSUM") as psS,
            tc.tile_pool(name="psM", bufs=1, space="PSUM") as psM,
            tc.tile_pool(name="psB", bufs=2, space="PSUM") as psB,
        ):
            def load_w(dram_ap, shape, dtype, tag):
                t = wp.tile(shape, dtype, tag=tag, name=tag)
                nc.sync.dma_start(out=t[:], in_=dram_ap)
                return t

            idsl = load_w(ids_loc[:, :], [P, NCHUNK], i32, "idsl")
            ident = load_w(ident_d[:, :], [P, P], f32, "ident")
            identb = load_w(identb_d[:, :], [P, P], bf16, "identb")
            halfi8 = load_w(halfi8_d[:, :], [P, P], f8, "halfi8")
            prepw = load_w(prep_w[:, :], [D, D], f32, "prepw")

            wnsb = [[load_w(wn_self_bf[m, l], [D, D], bf16, f"wnsb_{m}_{l}")
                     for l in range(2)] for m in range(NMP)]
            wnn1b = [load_w(wn_neigh1_bf[m], [D, D], bf16, f"wnn1b_{m}")
                     for m in range(NMP)]
            wes0 = [load_w(we_self[m, 0], [D, D], f32, f"wes0_{m}")
                    for m in range(NMP)]
            wnn0 = [load_w(wn_neigh[m, 0], [D, D], f32, f"wnn0_{m}")
                    for m in range(NMP)]
            wen0 = [load_w(we_neigh[m, 0], [D, D], f32, f"wen0_{m}")
                    for m in range(NMP)]
            epw = [load_w(ep_w[m], [DE, D], f32, f"epw_{m}") for m in range(NMP)]

            # ---- x0T = (feats[seed] @ prep_W)^T, fp32 then cast bf16 -------
            xf = wp.tile([P, NCHUNK, D], f32, tag="xf", name="xf")
            for c in range(NCHUNK):
                nc.gpsimd.indirect_dma_start(
                    out=xf[:, c, :], out_offset=None, in_=feats_sh[:, :],
                    in_offset=IOff(ap=idsl[:, c:c + 1], axis=0), oob_is_err=False)
            x0r = wp.tile([P, BCC], f32, tag="x0r", name="x0r")
            x0Tb = wp.tile([P, BCC], bf16, tag="x0Tb", name="x0Tb")
            for c in range(NCHUNK):
                cs = slice(c * P, (c + 1) * P)
                ps_xt = psM.tile([P, P], f32, tag="ps_misc", name="ps_xt")
                nc.tensor.transpose(out=ps_xt[:, :], in_=xf[:, c, :],
                                    identity=ident[:, :])
                nc.vector.tensor_copy(out=x0r[:, cs], in_=ps_xt[:, :])
                ps_xp = psM.tile([P, P], f32, tag="ps_misc", name="ps_xp")
                nc.tensor.matmul(out=ps_xp[:, :], lhsT=prepw[:, :],
                                 rhs=x0r[:, cs], start=True, stop=True)
                nc.vector.tensor_copy(out=x0Tb[:, cs], in_=ps_xp[:, :])

            # prep_W^T (shared by the PF folds)
            ps_f = psM.tile([P, P], f32, tag="ps_misc", name="ps_pwT")
            nc.tensor.transpose(out=ps_f[:, :], in_=prepw[:, :],
                                identity=ident[:, :])
            prepwT = wp.tile([P, P], f32, tag="prepwT", name="prepwT")
            nc.vector.tensor_copy(out=prepwT[:, :], in_=ps_f[:, :])

            for m in range(NMP):
                # ---- fold weights (fp32 matmuls, cast to bf16):
                #   A  = epW @ We_self0      (h1 edge-emb term)
                #   B  = epW @ Wn_neigh0 /32 (h0 m0 term; /32 = edge mean)
                #   PF = prep_W @ We_neigh0  (h1 pair-mean term; 0.5 in halfi8)
                ps_t = psM.tile([P, P], f32, tag="ps_misc", name="ps_epwT")
                nc.tensor.transpose(out=ps_t[0:D, 0:DE], in_=epw[m][:, :],
                                    identity=ident[0:DE, 0:DE])
                epwT = sp.tile([P, DE], f32, tag="epwT", name="epwT")
                nc.vector.tensor_copy(out=epwT[:, :], in_=ps_t[0:D, 0:DE])

                ps_a = psM.tile([P, P], f32, tag="ps_misc", name="ps_a")
                nc.tensor.matmul(out=ps_a[0:DE, :], lhsT=epwT[:, :],
                                 rhs=wes0[m][:, :], start=True, stop=True)
                a_t = wp.tile([DE, P], bf16, tag=f"a_t{m}", name=f"a_t{m}")
                nc.vector.tensor_copy(out=a_t[:, :], in_=ps_a[0:DE, :])

                ps_b = psM.tile([P, P], f32, tag="ps_misc", name="ps_b")
                nc.tensor.matmul(out=ps_b[0:DE, :], lhsT=epwT[:, :],
                                 rhs=wnn0[m][:, :], start=True, stop=True)
                b_t = wp.tile([DE, P], bf16, tag=f"b_t{m}", name=f"b_t{m}")
                nc.scalar.mul(out=b_t[:, :], in_=ps_b[0:DE, :], mul=1.0 / 32.0)

                ps_pf = psM.tile([P, P], f32, tag="ps_misc", name="ps_pf")
                nc.tensor.matmul(out=ps_pf[:, :], lhsT=prepwT[:, :],
                                 rhs=wen0[m][:, :], start=True, stop=True)
                pf_t = wp.tile([P, P], bf16, tag=f"pf_t{m}", name=f"pf_t{m}")
                nc.vector.tensor_copy(out=pf_t[:, :], in_=ps_pf[:, :])

                h0Tb = wp.tile([P, BCC], bf16, tag=f"h0Tb{m}", name=f"h0Tb{m}")

                for c in range(NCHUNK):
                    cs = slice(c * P, (c + 1) * P)
                    # ---- ONE fat gather: the chunk's 128 seed neighborhoods
                    gb = gp.tile([P, S, SLOT], u8, tag="gb", name="gb")
                    # flat 2-D APs: HW indirect DMA requires descriptor
                    # blocks == whole contiguous rows (3-D APs mispair)
                    nc.gpsimd.indirect_dma_start(
                        out=gb[:, :, :].opt(keep_dims=frozenset({0})),
                        out_offset=None,
                        in_=tab[m][:, :, :].opt(keep_dims=frozenset({0})),
                        in_offset=IOff(ap=idsl[:, c:c + 1], axis=0),
                        oob_is_err=False)

                    egT_sb = gp.tile([DE, S, D], bf16, tag="egT", name="egT")
                    sT_sb = gp.tile([P, S, D], bf16, tag="sTs", name="sTs")
                    mh_acc = sp.tile([P, D], bf16, tag="mha", name="mha")

                    for g in range(NGRP):
                        ps_h1 = psB.tile([P, 4 * P], f32, tag="ps_big",
                                         name="ps_h1")
                        for b in range(4):
                            j = 4 * g + b
                            emb_j = gb[:, j, 0:2 * DE].bitcast(bf16)
                            fu_j = gb[:, j, 2 * DE:2 * DE + D].bitcast(f8)
                            fv_j = gb[:, j, 2 * DE + D:SLOT].bitcast(f8)
                            # egT_j = emb_j^T (via identity matmul, 1cy bf16)
                            ps_eg = psS.tile([DE, D], f32, tag="ps_eg",
                                             name="ps_eg")
                            nc.tensor.matmul(out=ps_eg[:, :], lhsT=emb_j,
                                             rhs=identb[:, :], start=True,
                                             stop=True)
                            nc.scalar.copy(out=egT_sb[:, j, :], in_=ps_eg[:, :])
                            # sT_j = 0.5*(fu+fv)^T via two accumulating
                            # fp8 identity matmuls
                            ps_sT = psS.tile([P, D], f32, tag="ps_sT",
                                             name="ps_sT")
                            nc.tensor.matmul(out=ps_sT[:, :], lhsT=fu_j,
                                             rhs=halfi8[:, :], start=True,
                                             stop=False)
                            nc.tensor.matmul(out=ps_sT[:, :], lhsT=fv_j,
                                             rhs=halfi8[:, :], start=False,
                                             stop=True)
                            nc.vector.tensor_copy(out=sT_sb[:, j, :],
                                                  in_=ps_sT[:, :])

                        # h1T group [128 hidden, 4 slots x 128 seeds]
                        nc.tensor.matmul(out=ps_h1[:, :], lhsT=a_t[:, :],
                                         rhs=egT_sb[:, 4 * g:4 * g + 4, :],
                                         start=True, stop=False)
                        nc.tensor.matmul(out=ps_h1[:, :], lhsT=pf_t[:, :],
                                         rhs=sT_sb[:, 4 * g:4 * g + 4, :],
                                         start=False, stop=True)
                        h1r = tp.tile([P, 4, D], bf16, tag="h1r", name="h1r")
                        nc.scalar.activation(out=h1r[:, :, :], in_=ps_h1[:, :],
                                             func=Relu, scale=1.0 / 32.0)
                        # mh += sum over the 4 slots (2-level tree + running acc)
                        t1 = tp.tile([P, 2, D], bf16, tag="t1", name="t1")
                        nc.vector.tensor_add(out=t1[:, :, :],
                                             in0=h1r[:, 0:2, :],
                                             in1=h1r[:, 2:4, :])
                        if g == 0:
                            nc.vector.tensor_add(out=mh_acc[:, :],
                                                 in0=t1[:, 0, :],
                                                 in1=t1[:, 1, :])
                        else:
                            t2 = tp.tile([P, D], bf16, tag="t2", name="t2")
                            nc.vector.tensor_add(out=t2[:, :], in0=t1[:, 0, :],
                                                 in1=t1[:, 1, :])
                            nc.vector.tensor_add(out=mh_acc[:, :],
                                                 in0=mh_acc[:, :], in1=t2[:, :])

                    # ---- m0 = mean32 of emb (bf16 add-tree, /32 in b_t) ----
                    gbe = gb[:, :, 0:2 * DE].bitcast(bf16)  # [128, 32, 64]
                    m1 = tp.tile([P, 16, DE], bf16, tag="m1", name="m1")
                    nc.vector.tensor_add(out=m1[:, :, :], in0=gbe[:, 0:16, :],
                                         in1=gbe[:, 16:32, :])
                    m2 = tp.tile([P, 8, DE], bf16, tag="m2", name="m2")
                    nc.vector.tensor_add(out=m2[:, :, :], in0=m1[:, 0:8, :],
                                         in1=m1[:, 8:16, :])
                    m3 = tp.tile([P, 4, DE], bf16, tag="m3", name="m3")
                    nc.vector.tensor_add(out=m3[:, :, :], in0=m2[:, 0:4, :],
                                         in1=m2[:, 4:8, :])
                    m4 = tp.tile([P, 2, DE], bf16, tag="m4", name="m4")
                    nc.vector.tensor_add(out=m4[:, :, :], in0=m3[:, 0:2, :],
                                         in1=m3[:, 2:4, :])
                    m0rm = tp.tile([P, DE], bf16, tag="m0rm", name="m0rm")
                    nc.vector.tensor_add(out=m0rm[:, :], in0=m4[:, 0, :],
                                         in1=m4[:, 1, :])
                    ps_m0 = psM.tile([P, P], f32, tag="ps_misc", name="ps_m0")
                    nc.tensor.matmul(out=ps_m0[0:DE, :], lhsT=m0rm[:, :],
                                     rhs=identb[:, :], start=True, stop=True)
                    m0T = sp.tile([DE, P], bf16, tag="m0T", name="m0T")
                    nc.scalar.copy(out=m0T[:, :], in_=ps_m0[0:DE, :])

                    # ---- h0T chunk = relu(Wn_s0^T @ x0T + B^T @ m0T) -------
                    ps_h0 = psM.tile([P, P], f32, tag="ps_misc", name="ps_h0")
                    nc.tensor.matmul(out=ps_h0[:, :], lhsT=wnsb[m][0][:, :],
                                     rhs=x0Tb[:, cs], start=True, stop=False)
                    nc.tensor.matmul(out=ps_h0[:, :], lhsT=b_t[:, :],
                                     rhs=m0T[:, :], start=False, stop=True)
                    nc.scalar.activation(out=h0Tb[:, cs], in_=ps_h0[:, :],
                                         func=Relu)

                    # ---- o1T chunk = relu(Wn_s1^T @ h0T + Wn_n1^T @ mhT) ---
                    ps_o1 = psM.tile([P, P], f32, tag="ps_misc", name="ps_o1")
                    nc.tensor.matmul(out=ps_o1[:, :], lhsT=wnsb[m][1][:, :],
                                     rhs=h0Tb[:, cs], start=True, stop=False)
                    nc.tensor.matmul(out=ps_o1[:, :], lhsT=wnn1b[m][:, :],
                                     rhs=mh_acc[:, :], start=False, stop=True)
                    o1Tb = sp.tile([P, P], bf16, tag="o1Tb", name="o1Tb")
                    nc.scalar.activation(out=o1Tb[:, :], in_=ps_o1[:, :],
                                         func=Relu)

                    # ---- writeback: transpose to row-major, one DMA --------
                    ob = sp.tile([P, 2 * D], f32, tag="ob", name="ob")
                    ps_w = psM.tile([P, P], bf16, tag="ps_w", name="ps_w0")
                    nc.tensor.transpose(out=ps_w[:, :], in_=h0Tb[:, cs],
                                        identity=identb[:, :])
                    nc.vector.tensor_copy(out=ob[:, 0:D], in_=ps_w[:, :])
                    ps_w2 = psM.tile([P, P], bf16, tag="ps_w", name="ps_w1")
                    nc.tensor.transpose(out=ps_w2[:, :], in_=o1Tb[:, :],
                                        identity=identb[:, :])
                    nc.vector.tensor_copy(out=ob[:, D:2 * D], in_=ps_w2[:, :])
                    nc.sync.dma_start(out=out_t[m, cs, :], in_=ob[:, :])

    nc.compile()
    return nc


# ----------------------------------------------------------------------------
# Host-side input preparation (sharding + relayout + dtype casts)
# ----------------------------------------------------------------------------
_PERMS = {}


def make_in_maps(inputs, cfg, n_cores):
    import ml_dtypes
    bf16 = ml_dtypes.bfloat16
    f8 = ml_dtypes.float8_e4m3

    N, S = cfg["N"], cfg["S"]
    D, DE, NMP = cfg["D"], cfg["DE"], cfg["NMP"]
    BCC = cfg["BCC"]
    NSH = N // n_cores
    NCHUNK = BCC // P

    ids = np.asarray(inputs["ids"]).astype(np.int64)
    feats = np.asarray(inputs["feats"], dtype=np.float32)
    feats_f8 = feats.astype(f8).view(np.uint8)          # [N, 128] bytes

    # denormalized per-node tables: [N, S, 384] u8 = emb bf16 | fu fp8 | fv fp8
    tabs = []
    for mn in range(NMP):
        n2e = np.asarray(inputs[f"node2edge_idx_{mn}"]).astype(np.int64)
        adj = np.asarray(inputs[f"edge_node_adj_{mn}"]).astype(np.int64)
        emb_bf = np.asarray(
            inputs[f"edge_emb_{mn}"], dtype=np.float32).astype(bf16)
        t = np.empty((N, S, 2 * DE + 2 * D), np.uint8)
        t[:, :, 0:2 * DE] = emb_bf[n2e].view(np.uint8)
        t[:, :, 2 * DE:2 * DE + D] = feats_f8[adj[n2e, 0]]
        t[:, :, 2 * DE + D:] = feats_f8[adj[n2e, 1]]
        tabs.append(t)

    common = {
        "prep_w": np.asarray(inputs["prep_W"], dtype=np.float32),
        "ep_w": np.asarray(inputs["edge_prep_W"], dtype=np.float32),
        "we_self": np.asarray(inputs["We_self"], dtype=np.float32),
        "wn_neigh": np.asarray(inputs["Wn_neigh"], dtype=np.float32),
        "we_neigh": np.asarray(inputs["We_neigh"], dtype=np.float32),
        "wn_self_bf": np.asarray(inputs["Wn_self"]).astype(bf16),
        "wn_neigh1_bf": np.ascontiguousarray(
            np.asarray(inputs["Wn_neigh"])[:, 1].astype(bf16)),
        "ident": np.eye(P, dtype=np.float32),
        "ident_bf": np.eye(P, dtype=bf16),
        "half_ident_f8": (0.5 * np.eye(P)).astype(f8),
    }

    owner = ids // NSH
    in_maps = []
    _PERMS.clear()
    for core in range(n_cores):
        perm = np.where(owner == core)[0]
        assert len(perm) <= BCC, (
            f"core {core} got {len(perm)} seeds > capacity {BCC}")
        _PERMS[core] = perm
        loc = np.zeros(BCC, np.int32)
        loc[:len(perm)] = (ids[perm] - core * NSH).astype(np.int32)
        ids_loc = np.empty((P, NCHUNK), np.int32)
        for c in range(NCHUNK):
            ids_loc[:, c] = loc[c * P + np.arange(P)]
        m = dict(common)
        m["ids_loc"] = ids_loc
        m["feats_sh"] = feats[core * NSH:(core + 1) * NSH]
        for mn in range(NMP):
            m[f"tab_{mn}"] = tabs[mn][core * NSH:(core + 1) * NSH]
        in_maps.append(m)
    return in_maps


def assemble_output(results, cfg, n_cores):
    NMP, D = cfg["NMP"], cfg["D"]
    B = cfg["B"]
    out = np.empty((NMP, B, 2 * D), np.float32)
    for core in range(n_cores):
        perm = _PERMS[core]
        out[:, perm, :] = results[core]["out"][:, :len(perm), :]
    return out


FULL_CFG = dict(N=100000, E=400000, S=32, B=4096, BCC=640, D=128, DE=64, NMP=2)

_NC_CACHE = {}


def kernel(**inputs) -> np.ndarray:
    import sys
    for path in ("/opt/trn_rl_repo", "/root/.axon_site/_ro/trn_rl_repo"):
        if path not in sys.path:
            sys.path.append(path)
    from concourse.bass_utils import run_bass_kernel_spmd

    cfg = FULL_CFG
    n_cores = 8
    if "full" not in _NC_CACHE:
        _NC_CACHE["full"] = build_nc(cfg)
    nc = _NC_CACHE["full"]
    in_maps = make_in_maps(inputs, cfg, n_cores)
    res = run_bass_kernel_spmd(nc, in_maps, core_ids=list(range(n_cores)))
    return assemble_output(res.results, cfg, n_cores)
